# revision 36
# baseline (speedup 1.0000x reference)
"""BertSelfAttention (ALiBi-style additive bias) on 8 TRN2 NeuronCores.

Problem: B=4, S=1024, D=1024, H=16 heads (HD=64), fp32.
  qkv = hidden @ Wqkv_w.T + Wqkv_b
  scores = q @ k.T / sqrt(64) + bias ;  probs = softmax(scores) ; out = probs @ v

Sharding: 8 cores = 4 batches x 2 head-groups. Core c handles batch c//2 and
heads [ (c%2)*8, (c%2)*8+8 ).  Per-core shards are prepared host-side in the
layouts the TensorEngine wants (contraction dim on partitions) and cast to
bf16 (TensorE runs bf16 at full rate with fast weight loads; accumulation
stays fp32 in PSUM), so every device DMA is a contiguous, full-rate read:
  hw  [D, S+1536]  = [hidden[b].T | Wqkv rows for this core, transposed]
  wb  [1, 2*1536]  = [fused qkv bias slice | all-ones row]
  bT  [8, S, S]    = bias[b, h].T per head  (scores are computed transposed)
  idm [128, 128]   = identity (for the bias-add-by-matmul)
Device, per head: scoresT[k, q] = kT.T @ qT + biasT (identity-matmul
accumulated into the same PSUM tile), exp on ScalarE (no max-subtraction:
scores+bias <= ~10 so fp32 exp cannot overflow; large-negative ALiBi bias
cleanly underflows to 0), then outT[d, q] = [v | 1].T @ expT per 512-column
half, which also yields the softmax denominator in row 64.  Normalization =
broadcast the denominator over rows with a K=1 matmul, then fp32 DVE divide.
The host only re-transposes the per-core [512, S] result into (B, S, D).
"""

import numpy as np

import concourse.bacc as bacc
import concourse.bass as bass
import concourse.mybir as mybir
from concourse.tile import TileContext

B, S, D = 4, 1024, 1024
H = 16
HD = 64  # head dim
N_CORES = 8
HPC = 8  # heads per core
OC = 3 * HPC * HD  # 1536 fused-qkv output rows per core
F32 = mybir.dt.float32
BF16 = mybir.dt.bfloat16

KC = S // 128  # 8 key-token chunks of 128
TC_ = S // 128  # 8 token chunks of 128
DC = D // 128  # 8 contraction chunks of 128


def build_bass() -> bass.Bass:
    nc = bacc.Bacc()

    hw = nc.declare_dram_parameter("hw", [D, S + OC], BF16, isOutput=False)
    wb = nc.declare_dram_parameter("wb", [1, OC], BF16, isOutput=False)
    wbp = nc.declare_dram_parameter("wbp", [128, 12], F32, isOutput=False)
    bT = nc.declare_dram_parameter("bT", [HPC, S, S], BF16, isOutput=False)
    idm = nc.declare_dram_parameter("idm", [128, 128], BF16, isOutput=False)
    oT = nc.declare_dram_parameter("oT", [HPC * HD, S], F32, isOutput=True)

    with TileContext(nc) as tc:
        with (
            tc.tile_pool(name="const", bufs=1) as constp,
            tc.tile_pool(name="weights", bufs=1) as wp,
            tc.tile_pool(name="qk", bufs=1) as qkp,
            tc.tile_pool(name="vex", bufs=1) as vp,
            tc.tile_pool(name="bias", bufs=6) as btp,
            tc.tile_pool(name="exp", bufs=5) as ep,
            tc.tile_pool(name="outs", bufs=4) as op_,
            tc.tile_pool(name="ps_mm", bufs=2, space="PSUM") as ps_mm,
            tc.tile_pool(name="ps_sm", bufs=4, space="PSUM") as ps_sm,
        ):
            # --- constants -------------------------------------------------
            ident = constp.tile([128, 128], BF16)
            nc.sync.dma_start(out=ident[:], in_=idm[:])
            # fused qkv bias: wb_sb as a broadcast source for v's bias,
            # wbp_sb as per-partition [128,1] columns for q/k blocks
            wb_sb = constp.tile([1, OC], BF16)
            nc.sync.dma_start(out=wb_sb[:], in_=wb[:])
            wbp_sb = constp.tile([128, 12], F32)
            nc.sync.dma_start(out=wbp_sb[:], in_=wbp[:])
            wbv_b = constp.tile([128, HPC, HD], BF16)
            nc.gpsimd.partition_broadcast(
                wbv_b[:].rearrange("p h d -> p (h d)"),
                wb_sb[:, 2 * HPC * HD : 3 * HPC * HD],
            )

            # --- stage inputs ---------------------------------------------
            # one DMA per 128-row chunk carrying both hidden^T and W^T, so
            # each first consumer matmul waits on a single DMA semaphore
            hT_sb = []
            wT_sb = []
            for c in range(DC):
                hwt = wp.tile([128, S + OC], BF16, tag=f"hw{c}", name=f"hw{c}")
                if c == 0:
                    # split the first chunk so the first matmul (needs hT
                    # cols 0:512 + wT block 0) can start ~1us earlier
                    nc.sync.dma_start(
                        out=hwt[:, 0 : S + 128], in_=hw[0:128, 0 : S + 128]
                    )
                    nc.sync.dma_start(
                        out=hwt[:, S + 128 : S + OC],
                        in_=hw[0:128, S + 128 : S + OC],
                    )
                else:
                    nc.sync.dma_start(
                        out=hwt[:], in_=hw[c * 128 : (c + 1) * 128, :]
                    )
                hT_sb.append(hwt[:, 0:S])
                wT_sb.append(hwt[:, S : S + OC])

            # --- phase 1: fused QKV projection -----------------------------
            # qkT_sb[j][p, t]: j in 0..3 -> q rows (pre-scaled by 1/8),
            #                  j in 4..7 -> k rows. Row (j%4)*128+p = oc index.
            qk_sb = [
                qkp.tile([128, S], BF16, tag=f"qk{j}", name=f"qk{j}")
                for j in range(8)
            ]
            # v_sb[t][p, h, 0:64] = v head h, token t*128+p; [.., 64] = 1.0
            v_sb = [
                vp.tile([128, HPC, HD + 1], BF16, tag=f"vx{t}", name=f"v{t}")
                for t in range(TC_)
            ]

            # Emit in bands of up to 7 concurrent PSUM accumulation groups,
            # chunk-major, so PE has ~7 matmuls to run per arriving hw-chunk
            # DMA during the initial ramp instead of stalling per chunk.
            def qk_blk(j):
                ps = ps_mm.tile([128, S], F32, tag="mm", name=f"qkp{j}")

                def mm(c):
                    lw = wT_sb[c][:, j * 128 : (j + 1) * 128]
                    for half in range(2):
                        nc.tensor.matmul(
                            ps[:, half * 512 : (half + 1) * 512],
                            lw,
                            hT_sb[c][:, half * 512 : (half + 1) * 512],
                            start=(c == 0),
                            stop=(c == DC - 1),
                        )

                def fin():
                    # copy to SBUF, adding the per-partition qkv bias and
                    # folding the 1/sqrt(HD) score scale into q rows (DVE)
                    if j < 4:
                        nc.vector.tensor_scalar(
                            qk_sb[j][:], ps[:], wbp_sb[:, j : j + 1], 0.125,
                            op0=mybir.AluOpType.add, op1=mybir.AluOpType.mult,
                        )
                    else:
                        nc.vector.tensor_scalar_add(
                            qk_sb[j][:], ps[:], wbp_sb[:, j : j + 1]
                        )

                return mm, fin

            def v_blk(t):
                ps = ps_sm.tile([128, HPC * HD], F32, tag="sm", name=f"vps{t}")

                def mm(c):
                    nc.tensor.matmul(
                        ps[:],
                        hT_sb[c][:, t * 128 : (t + 1) * 128],
                        wT_sb[c][:, 2 * HPC * HD : 3 * HPC * HD],
                        start=(c == 0),
                        stop=(c == DC - 1),
                    )

                def fin():
                    nc.vector.tensor_tensor(
                        v_sb[t][:, :, 0:HD],
                        ps[:].rearrange("p (h d) -> p h d", h=HPC),
                        wbv_b[:],
                        op=mybir.AluOpType.add,
                    )
                    nc.scalar.activation(
                        v_sb[t][:, :, HD : HD + 1],
                        v_sb[t][:, :, 0:1],
                        mybir.ActivationFunctionType.Identity,
                        scale=0.0,
                        bias=1.0,
                    )

                return mm, fin

            bands = [
                [qk_blk(0), qk_blk(4), v_blk(0), v_blk(1), v_blk(2)],
                [qk_blk(1), qk_blk(5), v_blk(3), v_blk(4), v_blk(5)],
                [qk_blk(2), qk_blk(6), v_blk(6), v_blk(7)],
                [qk_blk(3), qk_blk(7)],
            ]
            for band in bands:
                for c in range(DC):
                    for mm, _ in band:
                        mm(c)
                for _, fin in band:
                    fin()

            # --- phase 2: attention ----------------------------------------
            # Software-pipelined across (head, k-chunk) items: the AV matmuls
            # for item i are emitted DEPTH items late so the in-order PE
            # stream never stalls waiting on that item's exp.
            DEPTH = 3
            items = [(h, kc) for h in range(HPC) for kc in range(KC)]
            ets: dict[int, object] = {}
            pos_map: dict[int, list] = {}

            def emit_front(i):
                h, kc = items[i]
                j, po = h // 2, (h % 2) * 64
                qT = qk_sb[j][po : po + 64, :]  # [64, S] (already /8)
                kT = qk_sb[4 + j][po : po + 64, :]  # [64, S]
                bt = btp.tile([128, S], BF16, tag="bt", name=f"bt{i}")
                nc.sync.dma_start(
                    out=bt[:], in_=bT[h, kc * 128 : (kc + 1) * 128, :]
                )
                ps = ps_mm.tile([128, S], F32, tag="mm", name=f"s{i}")
                # scoresT[k, q] = k @ q.T  (contraction over head dim)
                for half in range(2):
                    nc.tensor.matmul(
                        ps[:, half * 512 : (half + 1) * 512],
                        kT[:, kc * 128 : (kc + 1) * 128],
                        qT[:, half * 512 : (half + 1) * 512],
                        start=True,
                        stop=False,
                    )
                # += biasT via identity matmul (I.T @ bt = bt)
                for half in range(2):
                    nc.tensor.matmul(
                        ps[:, half * 512 : (half + 1) * 512],
                        ident[:],
                        bt[:, half * 512 : (half + 1) * 512],
                        start=False,
                        stop=True,
                    )
                et = ep.tile([128, S], BF16, tag="et", name=f"et{i}")
                nc.scalar.activation(et[:], ps[:], mybir.ActivationFunctionType.Exp)
                ets[i] = et

            def emit_back(i):
                h, kc = items[i]
                if h not in pos_map:
                    # [65, 512] 1-bank output tiles: rows 0..63 = outT,
                    # row 64 = sum of exp
                    pos_map[h] = [
                        ps_sm.tile([HD + 1, 512], F32, tag="sm", name=f"po{h}_{k}")
                        for k in range(2)
                    ]
                # outT[d,q] += v_ext.T @ expT ; row 64 = sum(exp)
                et = ets.pop(i)
                for half in range(2):
                    nc.tensor.matmul(
                        pos_map[h][half][:],
                        v_sb[kc][:, h, :],
                        et[:, half * 512 : (half + 1) * 512],
                        start=(kc == 0),
                        stop=(kc == KC - 1),
                    )
                if kc == KC - 1:
                    emit_tail(h)

            def emit_tail(h):
                # normalize: out[d,q] * (1/sum[q]) per half.  1/sum via
                # reciprocal_approx_fast (18 bits; sums are benign), broadcast
                # along partitions on the idle GpSimd, multiply on DVE.
                for half in range(2):
                    p = pos_map[h][half]
                    smf = op_.tile([1, 512], F32, tag="smf")
                    nc.vector.tensor_copy(smf[:], p[HD : HD + 1, :])
                    rcf = op_.tile([1, 512], F32, tag="rcf")
                    nc.vector.reciprocal_approx_fast(rcf[:], smf[:])
                    rb = op_.tile([HD, 512], F32, tag="rb")
                    nc.gpsimd.partition_broadcast(rb[:], rcf[:])
                    ot = op_.tile([HD, 512], F32, tag="ot")
                    nc.vector.tensor_tensor(
                        ot[:], p[0:HD, :], rb[:], op=mybir.AluOpType.mult
                    )
                    nc.sync.dma_start(
                        out=oT[
                            h * HD : (h + 1) * HD, half * 512 : (half + 1) * 512
                        ],
                        in_=ot[:],
                    )

            for i in range(len(items)):
                emit_front(i)
                if i >= DEPTH:
                    emit_back(i - DEPTH)
            for i in range(len(items) - DEPTH, len(items)):
                emit_back(i)

    # Bacc defers register allocation to its compile() pass, which only runs
    # in finalize(); run_bass_via_pjrt ships the BIR as-is, so finalize here.
    nc.finalize()
    return nc


def shard_inputs(hidden_states, bias, Wqkv_w, Wqkv_b):
    """Slice + lay out the full inputs into 8 per-core input maps."""
    import ml_dtypes

    bf16 = ml_dtypes.bfloat16
    hidden_states = np.asarray(hidden_states, dtype=np.float32)
    bias = np.asarray(bias, dtype=np.float32)
    Wqkv_w = np.asarray(Wqkv_w, dtype=np.float32)
    Wqkv_b = np.asarray(Wqkv_b, dtype=np.float32)

    in_maps = []
    eye = np.eye(128, dtype=bf16)
    for c in range(N_CORES):
        b, hs = c // 2, (c % 2) * HPC
        rows = np.concatenate(
            [np.arange(g * D + hs * HD, g * D + (hs + HPC) * HD) for g in range(3)]
        )
        wb2 = Wqkv_b[rows][None, :].astype(bf16)
        wbp2 = np.ascontiguousarray(
            Wqkv_b[rows].reshape(12, 128).T
        ).astype(np.float32)
        in_maps.append(
            {
                "hw": np.concatenate(
                    [hidden_states[b].T, Wqkv_w[rows].T], axis=1
                ).astype(bf16),
                "wb": wb2,
                "wbp": wbp2,
                "bT": np.ascontiguousarray(
                    bias[b, hs : hs + HPC].transpose(0, 2, 1)
                ).astype(bf16),
                "idm": eye,
            }
        )
    return in_maps


_CACHED_NC = None


def kernel(hidden_states, bias, Wqkv_w, Wqkv_b):
    from concourse.bass_utils import run_bass_kernel_spmd

    global _CACHED_NC
    if _CACHED_NC is None:
        _CACHED_NC = build_bass()
    in_maps = shard_inputs(hidden_states, bias, Wqkv_w, Wqkv_b)
    res = run_bass_kernel_spmd(_CACHED_NC, in_maps, core_ids=list(range(N_CORES)))
    out = np.empty((B, S, D), dtype=np.float32)
    for c in range(N_CORES):
        b, hs = c // 2, (c % 2) * HPC
        out[b, :, hs * HD : (hs + HPC) * HD] = res.results[c]["oT"].T
    return out


# revision 38
# speedup vs baseline: 1.0028x; 1.0028x over previous
"""BertSelfAttention (ALiBi-style additive bias) on 8 TRN2 NeuronCores.

Problem: B=4, S=1024, D=1024, H=16 heads (HD=64), fp32.
  qkv = hidden @ Wqkv_w.T + Wqkv_b
  scores = q @ k.T / sqrt(64) + bias ;  probs = softmax(scores) ; out = probs @ v

Sharding: 8 cores = 4 batches x 2 head-groups. Core c handles batch c//2 and
heads [ (c%2)*8, (c%2)*8+8 ).  Per-core shards are prepared host-side in the
layouts the TensorEngine wants (contraction dim on partitions) and cast to
bf16 (TensorE runs bf16 at full rate with fast weight loads; accumulation
stays fp32 in PSUM), so every device DMA is a contiguous, full-rate read:
  hw  [D, S+1536]  = [hidden[b].T | Wqkv rows for this core, transposed]
  wb  [1, 2*1536]  = [fused qkv bias slice | all-ones row]
  bT  [8, S, S]    = bias[b, h].T per head  (scores are computed transposed)
  idm [128, 128]   = identity (for the bias-add-by-matmul)
Device, per head: scoresT[k, q] = kT.T @ qT + biasT (identity-matmul
accumulated into the same PSUM tile), exp on ScalarE (no max-subtraction:
scores+bias <= ~10 so fp32 exp cannot overflow; large-negative ALiBi bias
cleanly underflows to 0), then outT[d, q] = [v | 1].T @ expT per 512-column
half, which also yields the softmax denominator in row 64.  Normalization =
broadcast the denominator over rows with a K=1 matmul, then fp32 DVE divide.
The host only re-transposes the per-core [512, S] result into (B, S, D).
"""

import numpy as np

import concourse.bacc as bacc
import concourse.bass as bass
import concourse.mybir as mybir
from concourse.tile import TileContext

B, S, D = 4, 1024, 1024
H = 16
HD = 64  # head dim
N_CORES = 8
HPC = 8  # heads per core
OC = 3 * HPC * HD  # 1536 fused-qkv output rows per core
F32 = mybir.dt.float32
BF16 = mybir.dt.bfloat16

KC = S // 128  # 8 key-token chunks of 128
TC_ = S // 128  # 8 token chunks of 128
DC = D // 128  # 8 contraction chunks of 128


def build_bass() -> bass.Bass:
    nc = bacc.Bacc()

    hw = nc.declare_dram_parameter("hw", [D, S + OC], BF16, isOutput=False)
    wb = nc.declare_dram_parameter("wb", [1, OC], BF16, isOutput=False)
    wbp = nc.declare_dram_parameter("wbp", [128, 12], F32, isOutput=False)
    bT = nc.declare_dram_parameter("bT", [HPC, S, S], BF16, isOutput=False)
    idm = nc.declare_dram_parameter("idm", [128, 128], BF16, isOutput=False)
    oT = nc.declare_dram_parameter("oT", [HPC * HD, S], F32, isOutput=True)

    with TileContext(nc) as tc:
        with (
            tc.tile_pool(name="const", bufs=1) as constp,
            tc.tile_pool(name="weights", bufs=1) as wp,
            tc.tile_pool(name="qk", bufs=1) as qkp,
            tc.tile_pool(name="vex", bufs=1) as vp,
            tc.tile_pool(name="bias", bufs=6) as btp,
            tc.tile_pool(name="exp", bufs=5) as ep,
            tc.tile_pool(name="outs", bufs=4) as op_,
            tc.tile_pool(name="ps_mm", bufs=2, space="PSUM") as ps_mm,
            tc.tile_pool(name="ps_sm", bufs=4, space="PSUM") as ps_sm,
        ):
            # --- constants -------------------------------------------------
            ident = constp.tile([128, 128], BF16)
            nc.sync.dma_start(out=ident[:], in_=idm[:])
            # fused qkv bias: wb_sb as a broadcast source for v's bias,
            # wbp_sb as per-partition [128,1] columns for q/k blocks
            wb_sb = constp.tile([1, OC], BF16)
            nc.sync.dma_start(out=wb_sb[:], in_=wb[:])
            wbp_sb = constp.tile([128, 12], F32)
            nc.sync.dma_start(out=wbp_sb[:], in_=wbp[:])
            wbv_b = constp.tile([128, HPC, HD], BF16)
            nc.gpsimd.partition_broadcast(
                wbv_b[:].rearrange("p h d -> p (h d)"),
                wb_sb[:, 2 * HPC * HD : 3 * HPC * HD],
            )

            # --- stage inputs ---------------------------------------------
            # one DMA per 128-row chunk carrying both hidden^T and W^T, so
            # each first consumer matmul waits on a single DMA semaphore
            hT_sb = []
            wT_sb = []
            for c in range(DC):
                hwt = wp.tile([128, S + OC], BF16, tag=f"hw{c}", name=f"hw{c}")
                nc.sync.dma_start(out=hwt[:], in_=hw[c * 128 : (c + 1) * 128, :])
                hT_sb.append(hwt[:, 0:S])
                wT_sb.append(hwt[:, S : S + OC])

            # --- phase 1: fused QKV projection -----------------------------
            # qkT_sb[j][p, t]: j in 0..3 -> q rows (pre-scaled by 1/8),
            #                  j in 4..7 -> k rows. Row (j%4)*128+p = oc index.
            qk_sb = [
                qkp.tile([128, S], BF16, tag=f"qk{j}", name=f"qk{j}")
                for j in range(8)
            ]
            # v_sb[t][p, h, 0:64] = v head h, token t*128+p; [.., 64] = 1.0
            v_sb = [
                vp.tile([128, HPC, HD + 1], BF16, tag=f"vx{t}", name=f"v{t}")
                for t in range(TC_)
            ]

            # Emit in bands of up to 7 concurrent PSUM accumulation groups,
            # chunk-major, so PE has ~7 matmuls to run per arriving hw-chunk
            # DMA during the initial ramp instead of stalling per chunk.
            def qk_blk(j):
                ps = ps_mm.tile([128, S], F32, tag="mm", name=f"qkp{j}")

                def mm(c):
                    lw = wT_sb[c][:, j * 128 : (j + 1) * 128]
                    for half in range(2):
                        nc.tensor.matmul(
                            ps[:, half * 512 : (half + 1) * 512],
                            lw,
                            hT_sb[c][:, half * 512 : (half + 1) * 512],
                            start=(c == 0),
                            stop=(c == DC - 1),
                        )

                def fin():
                    # copy to SBUF, adding the per-partition qkv bias and
                    # folding the 1/sqrt(HD) score scale into q rows (DVE)
                    if j < 4:
                        nc.vector.tensor_scalar(
                            qk_sb[j][:], ps[:], wbp_sb[:, j : j + 1], 0.125,
                            op0=mybir.AluOpType.add, op1=mybir.AluOpType.mult,
                        )
                    else:
                        nc.vector.tensor_scalar_add(
                            qk_sb[j][:], ps[:], wbp_sb[:, j : j + 1]
                        )

                return mm, fin

            def v_blk(t):
                ps = ps_sm.tile([128, HPC * HD], F32, tag="sm", name=f"vps{t}")

                def mm(c):
                    nc.tensor.matmul(
                        ps[:],
                        hT_sb[c][:, t * 128 : (t + 1) * 128],
                        wT_sb[c][:, 2 * HPC * HD : 3 * HPC * HD],
                        start=(c == 0),
                        stop=(c == DC - 1),
                    )

                def fin():
                    nc.vector.tensor_tensor(
                        v_sb[t][:, :, 0:HD],
                        ps[:].rearrange("p (h d) -> p h d", h=HPC),
                        wbv_b[:],
                        op=mybir.AluOpType.add,
                    )
                    nc.scalar.activation(
                        v_sb[t][:, :, HD : HD + 1],
                        v_sb[t][:, :, 0:1],
                        mybir.ActivationFunctionType.Identity,
                        scale=0.0,
                        bias=1.0,
                    )

                return mm, fin

            bands = [
                [qk_blk(0), qk_blk(4), v_blk(0), v_blk(1), v_blk(2)],
                [qk_blk(1), qk_blk(5), v_blk(3), v_blk(4), v_blk(5)],
                [qk_blk(2), qk_blk(6), v_blk(6), v_blk(7)],
                [qk_blk(3), qk_blk(7)],
            ]
            for band in bands:
                for c in range(DC):
                    for mm, _ in band:
                        mm(c)
                for _, fin in band:
                    fin()

            # --- phase 2: attention ----------------------------------------
            # Software-pipelined across (head, k-chunk) items: the AV matmuls
            # for item i are emitted DEPTH items late so the in-order PE
            # stream never stalls waiting on that item's exp.
            DEPTH = 2
            items = [(h, kc) for h in range(HPC) for kc in range(KC)]
            ets: dict[int, object] = {}
            pos_map: dict[int, list] = {}

            def emit_front(i):
                h, kc = items[i]
                j, po = h // 2, (h % 2) * 64
                qT = qk_sb[j][po : po + 64, :]  # [64, S] (already /8)
                kT = qk_sb[4 + j][po : po + 64, :]  # [64, S]
                bt = btp.tile([128, S], BF16, tag="bt", name=f"bt{i}")
                nc.sync.dma_start(
                    out=bt[:], in_=bT[h, kc * 128 : (kc + 1) * 128, :]
                )
                ps = ps_mm.tile([128, S], F32, tag="mm", name=f"s{i}")
                # scoresT[k, q] = k @ q.T  (contraction over head dim)
                for half in range(2):
                    nc.tensor.matmul(
                        ps[:, half * 512 : (half + 1) * 512],
                        kT[:, kc * 128 : (kc + 1) * 128],
                        qT[:, half * 512 : (half + 1) * 512],
                        start=True,
                        stop=False,
                    )
                # += biasT via identity matmul (I.T @ bt = bt)
                for half in range(2):
                    nc.tensor.matmul(
                        ps[:, half * 512 : (half + 1) * 512],
                        ident[:],
                        bt[:, half * 512 : (half + 1) * 512],
                        start=False,
                        stop=True,
                    )
                et = ep.tile([128, S], BF16, tag="et", name=f"et{i}")
                nc.scalar.activation(et[:], ps[:], mybir.ActivationFunctionType.Exp)
                ets[i] = et

            def emit_back(i):
                h, kc = items[i]
                if h not in pos_map:
                    # [65, 512] 1-bank output tiles: rows 0..63 = outT,
                    # row 64 = sum of exp
                    pos_map[h] = [
                        ps_sm.tile([HD + 1, 512], F32, tag="sm", name=f"po{h}_{k}")
                        for k in range(2)
                    ]
                # outT[d,q] += v_ext.T @ expT ; row 64 = sum(exp)
                et = ets.pop(i)
                for half in range(2):
                    nc.tensor.matmul(
                        pos_map[h][half][:],
                        v_sb[kc][:, h, :],
                        et[:, half * 512 : (half + 1) * 512],
                        start=(kc == 0),
                        stop=(kc == KC - 1),
                    )
                if kc == KC - 1:
                    emit_tail(h)

            def emit_tail(h):
                # normalize: out[d,q] * (1/sum[q]) per half.  1/sum via
                # reciprocal_approx_fast (18 bits; sums are benign), broadcast
                # along partitions on the idle GpSimd, multiply on DVE.
                for half in range(2):
                    p = pos_map[h][half]
                    smf = op_.tile([1, 512], F32, tag="smf")
                    nc.scalar.activation(
                        smf[:], p[HD : HD + 1, :],
                        mybir.ActivationFunctionType.Copy,
                    )
                    rcf = op_.tile([1, 512], F32, tag="rcf")
                    nc.vector.reciprocal_approx_fast(rcf[:], smf[:])
                    rb = op_.tile([HD, 512], F32, tag="rb")
                    nc.gpsimd.partition_broadcast(rb[:], rcf[:])
                    ot = op_.tile([HD, 512], F32, tag="ot")
                    nc.vector.tensor_tensor(
                        ot[:], p[0:HD, :], rb[:], op=mybir.AluOpType.mult
                    )
                    nc.sync.dma_start(
                        out=oT[
                            h * HD : (h + 1) * HD, half * 512 : (half + 1) * 512
                        ],
                        in_=ot[:],
                    )

            for i in range(len(items)):
                emit_front(i)
                if i >= DEPTH:
                    emit_back(i - DEPTH)
            for i in range(len(items) - DEPTH, len(items)):
                emit_back(i)

    # Bacc defers register allocation to its compile() pass, which only runs
    # in finalize(); run_bass_via_pjrt ships the BIR as-is, so finalize here.
    nc.finalize()
    return nc


def shard_inputs(hidden_states, bias, Wqkv_w, Wqkv_b):
    """Slice + lay out the full inputs into 8 per-core input maps."""
    import ml_dtypes

    bf16 = ml_dtypes.bfloat16
    hidden_states = np.asarray(hidden_states, dtype=np.float32)
    bias = np.asarray(bias, dtype=np.float32)
    Wqkv_w = np.asarray(Wqkv_w, dtype=np.float32)
    Wqkv_b = np.asarray(Wqkv_b, dtype=np.float32)

    in_maps = []
    eye = np.eye(128, dtype=bf16)
    for c in range(N_CORES):
        b, hs = c // 2, (c % 2) * HPC
        rows = np.concatenate(
            [np.arange(g * D + hs * HD, g * D + (hs + HPC) * HD) for g in range(3)]
        )
        wb2 = Wqkv_b[rows][None, :].astype(bf16)
        wbp2 = np.ascontiguousarray(
            Wqkv_b[rows].reshape(12, 128).T
        ).astype(np.float32)
        in_maps.append(
            {
                "hw": np.concatenate(
                    [hidden_states[b].T, Wqkv_w[rows].T], axis=1
                ).astype(bf16),
                "wb": wb2,
                "wbp": wbp2,
                "bT": np.ascontiguousarray(
                    bias[b, hs : hs + HPC].transpose(0, 2, 1)
                ).astype(bf16),
                "idm": eye,
            }
        )
    return in_maps


_CACHED_NC = None


def kernel(hidden_states, bias, Wqkv_w, Wqkv_b):
    from concourse.bass_utils import run_bass_kernel_spmd

    global _CACHED_NC
    if _CACHED_NC is None:
        _CACHED_NC = build_bass()
    in_maps = shard_inputs(hidden_states, bias, Wqkv_w, Wqkv_b)
    res = run_bass_kernel_spmd(_CACHED_NC, in_maps, core_ids=list(range(N_CORES)))
    out = np.empty((B, S, D), dtype=np.float32)
    for c in range(N_CORES):
        b, hs = c // 2, (c % 2) * HPC
        out[b, :, hs * HD : (hs + HPC) * HD] = res.results[c]["oT"].T
    return out


# revision 40
# speedup vs baseline: 1.0046x; 1.0018x over previous
"""BertSelfAttention (ALiBi-style additive bias) on 8 TRN2 NeuronCores.

Problem: B=4, S=1024, D=1024, H=16 heads (HD=64), fp32.
  qkv = hidden @ Wqkv_w.T + Wqkv_b
  scores = q @ k.T / sqrt(64) + bias ;  probs = softmax(scores) ; out = probs @ v

Sharding: 8 cores = 4 batches x 2 head-groups. Core c handles batch c//2 and
heads [ (c%2)*8, (c%2)*8+8 ).  Per-core shards are prepared host-side in the
layouts the TensorEngine wants (contraction dim on partitions) and cast to
bf16 (TensorE runs bf16 at full rate with fast weight loads; accumulation
stays fp32 in PSUM), so every device DMA is a contiguous, full-rate read:
  hw  [D, S+1536]  = [hidden[b].T | Wqkv rows for this core, transposed]
  wb  [1, 2*1536]  = [fused qkv bias slice | all-ones row]
  bT  [8, S, S]    = bias[b, h].T per head  (scores are computed transposed)
  idm [128, 128]   = identity (for the bias-add-by-matmul)
Device, per head: scoresT[k, q] = kT.T @ qT + biasT (identity-matmul
accumulated into the same PSUM tile), exp on ScalarE (no max-subtraction:
scores+bias <= ~10 so fp32 exp cannot overflow; large-negative ALiBi bias
cleanly underflows to 0), then outT[d, q] = [v | 1].T @ expT per 512-column
half, which also yields the softmax denominator in row 64.  Normalization =
broadcast the denominator over rows with a K=1 matmul, then fp32 DVE divide.
The host only re-transposes the per-core [512, S] result into (B, S, D).
"""

import numpy as np

import concourse.bacc as bacc
import concourse.bass as bass
import concourse.mybir as mybir
from concourse.tile import TileContext

B, S, D = 4, 1024, 1024
H = 16
HD = 64  # head dim
N_CORES = 8
HPC = 8  # heads per core
OC = 3 * HPC * HD  # 1536 fused-qkv output rows per core
F32 = mybir.dt.float32
BF16 = mybir.dt.bfloat16

KC = S // 128  # 8 key-token chunks of 128
TC_ = S // 128  # 8 token chunks of 128
DC = D // 128  # 8 contraction chunks of 128


def build_bass() -> bass.Bass:
    nc = bacc.Bacc()

    hw = nc.declare_dram_parameter("hw", [D, S + OC], BF16, isOutput=False)
    wb = nc.declare_dram_parameter("wb", [1, OC], BF16, isOutput=False)
    wbp = nc.declare_dram_parameter("wbp", [128, 12], F32, isOutput=False)
    bT = nc.declare_dram_parameter("bT", [HPC, S, S], BF16, isOutput=False)
    idm = nc.declare_dram_parameter("idm", [128, 128], BF16, isOutput=False)
    oT = nc.declare_dram_parameter("oT", [HPC * HD, S], F32, isOutput=True)

    with TileContext(nc) as tc:
        with (
            tc.tile_pool(name="const", bufs=1) as constp,
            tc.tile_pool(name="weights", bufs=1) as wp,
            tc.tile_pool(name="qk", bufs=1) as qkp,
            tc.tile_pool(name="vex", bufs=1) as vp,
            tc.tile_pool(name="bias", bufs=8) as btp,
            tc.tile_pool(name="exp", bufs=5) as ep,
            tc.tile_pool(name="outs", bufs=4) as op_,
            tc.tile_pool(name="ps_mm", bufs=2, space="PSUM") as ps_mm,
            tc.tile_pool(name="ps_sm", bufs=4, space="PSUM") as ps_sm,
        ):
            # --- constants -------------------------------------------------
            ident = constp.tile([128, 128], BF16)
            nc.sync.dma_start(out=ident[:], in_=idm[:])
            # fused qkv bias: wb_sb as a broadcast source for v's bias,
            # wbp_sb as per-partition [128,1] columns for q/k blocks
            wb_sb = constp.tile([1, OC], BF16)
            nc.sync.dma_start(out=wb_sb[:], in_=wb[:])
            wbp_sb = constp.tile([128, 12], F32)
            nc.sync.dma_start(out=wbp_sb[:], in_=wbp[:])
            wbv_b = constp.tile([128, HPC, HD], BF16)
            nc.gpsimd.partition_broadcast(
                wbv_b[:].rearrange("p h d -> p (h d)"),
                wb_sb[:, 2 * HPC * HD : 3 * HPC * HD],
            )

            # --- stage inputs ---------------------------------------------
            # one DMA per 128-row chunk carrying both hidden^T and W^T, so
            # each first consumer matmul waits on a single DMA semaphore
            hT_sb = []
            wT_sb = []
            for c in range(DC):
                hwt = wp.tile([128, S + OC], BF16, tag=f"hw{c}", name=f"hw{c}")
                nc.sync.dma_start(out=hwt[:], in_=hw[c * 128 : (c + 1) * 128, :])
                hT_sb.append(hwt[:, 0:S])
                wT_sb.append(hwt[:, S : S + OC])

            # --- phase 1: fused QKV projection -----------------------------
            # qkT_sb[j][p, t]: j in 0..3 -> q rows (pre-scaled by 1/8),
            #                  j in 4..7 -> k rows. Row (j%4)*128+p = oc index.
            qk_sb = [
                qkp.tile([128, S], BF16, tag=f"qk{j}", name=f"qk{j}")
                for j in range(8)
            ]
            # v_sb[t][p, h, 0:64] = v head h, token t*128+p; [.., 64] = 1.0
            v_sb = [
                vp.tile([128, HPC, HD + 1], BF16, tag=f"vx{t}", name=f"v{t}")
                for t in range(TC_)
            ]

            # Emit in bands of up to 7 concurrent PSUM accumulation groups,
            # chunk-major, so PE has ~7 matmuls to run per arriving hw-chunk
            # DMA during the initial ramp instead of stalling per chunk.
            def qk_blk(j):
                ps = ps_mm.tile([128, S], F32, tag="mm", name=f"qkp{j}")

                def mm(c):
                    lw = wT_sb[c][:, j * 128 : (j + 1) * 128]
                    for half in range(2):
                        nc.tensor.matmul(
                            ps[:, half * 512 : (half + 1) * 512],
                            lw,
                            hT_sb[c][:, half * 512 : (half + 1) * 512],
                            start=(c == 0),
                            stop=(c == DC - 1),
                        )

                def fin():
                    # copy to SBUF, adding the per-partition qkv bias and
                    # folding the 1/sqrt(HD) score scale into q rows (DVE)
                    if j < 4:
                        nc.vector.tensor_scalar(
                            qk_sb[j][:], ps[:], wbp_sb[:, j : j + 1], 0.125,
                            op0=mybir.AluOpType.add, op1=mybir.AluOpType.mult,
                        )
                    else:
                        nc.vector.tensor_scalar_add(
                            qk_sb[j][:], ps[:], wbp_sb[:, j : j + 1]
                        )

                return mm, fin

            def v_blk(t):
                ps = ps_sm.tile([128, HPC * HD], F32, tag="sm", name=f"vps{t}")

                def mm(c):
                    nc.tensor.matmul(
                        ps[:],
                        hT_sb[c][:, t * 128 : (t + 1) * 128],
                        wT_sb[c][:, 2 * HPC * HD : 3 * HPC * HD],
                        start=(c == 0),
                        stop=(c == DC - 1),
                    )

                def fin():
                    nc.vector.tensor_tensor(
                        v_sb[t][:, :, 0:HD],
                        ps[:].rearrange("p (h d) -> p h d", h=HPC),
                        wbv_b[:],
                        op=mybir.AluOpType.add,
                    )
                    nc.scalar.activation(
                        v_sb[t][:, :, HD : HD + 1],
                        v_sb[t][:, :, 0:1],
                        mybir.ActivationFunctionType.Identity,
                        scale=0.0,
                        bias=1.0,
                    )

                return mm, fin

            bands = [
                [qk_blk(0), qk_blk(4), v_blk(0), v_blk(1), v_blk(2)],
                [qk_blk(1), qk_blk(5), v_blk(3), v_blk(4), v_blk(5)],
                [qk_blk(2), qk_blk(6), v_blk(6), v_blk(7)],
                [qk_blk(3), qk_blk(7)],
            ]
            for band in bands:
                for c in range(DC):
                    for mm, _ in band:
                        mm(c)
                for _, fin in band:
                    fin()

            # --- phase 2: attention ----------------------------------------
            # Software-pipelined across (head, k-chunk) items: the AV matmuls
            # for item i are emitted DEPTH items late so the in-order PE
            # stream never stalls waiting on that item's exp.
            DEPTH = 3
            items = [(h, kc) for h in range(HPC) for kc in range(KC)]
            ets: dict[int, object] = {}
            pos_map: dict[int, list] = {}

            def emit_front(i):
                h, kc = items[i]
                j, po = h // 2, (h % 2) * 64
                qT = qk_sb[j][po : po + 64, :]  # [64, S] (already /8)
                kT = qk_sb[4 + j][po : po + 64, :]  # [64, S]
                bt = btp.tile([128, S], BF16, tag="bt", name=f"bt{i}")
                nc.sync.dma_start(
                    out=bt[:], in_=bT[h, kc * 128 : (kc + 1) * 128, :]
                )
                ps = ps_mm.tile([128, S], F32, tag="mm", name=f"s{i}")
                # scoresT[k, q] = k @ q.T  (contraction over head dim)
                for half in range(2):
                    nc.tensor.matmul(
                        ps[:, half * 512 : (half + 1) * 512],
                        kT[:, kc * 128 : (kc + 1) * 128],
                        qT[:, half * 512 : (half + 1) * 512],
                        start=True,
                        stop=False,
                    )
                # += biasT via identity matmul (I.T @ bt = bt)
                for half in range(2):
                    nc.tensor.matmul(
                        ps[:, half * 512 : (half + 1) * 512],
                        ident[:],
                        bt[:, half * 512 : (half + 1) * 512],
                        start=False,
                        stop=True,
                    )
                et = ep.tile([128, S], BF16, tag="et", name=f"et{i}")
                nc.scalar.activation(et[:], ps[:], mybir.ActivationFunctionType.Exp)
                ets[i] = et

            def emit_back(i):
                h, kc = items[i]
                if h not in pos_map:
                    # [65, 512] 1-bank output tiles: rows 0..63 = outT,
                    # row 64 = sum of exp
                    pos_map[h] = [
                        ps_sm.tile([HD + 1, 512], F32, tag="sm", name=f"po{h}_{k}")
                        for k in range(2)
                    ]
                # outT[d,q] += v_ext.T @ expT ; row 64 = sum(exp)
                et = ets.pop(i)
                for half in range(2):
                    nc.tensor.matmul(
                        pos_map[h][half][:],
                        v_sb[kc][:, h, :],
                        et[:, half * 512 : (half + 1) * 512],
                        start=(kc == 0),
                        stop=(kc == KC - 1),
                    )
                if kc == KC - 1:
                    emit_tail(h)

            def emit_tail(h):
                # normalize: out[d,q] * (1/sum[q]) per half.  1/sum via
                # reciprocal_approx_fast (18 bits; sums are benign), broadcast
                # along partitions on the idle GpSimd, multiply on DVE.
                for half in range(2):
                    p = pos_map[h][half]
                    smf = op_.tile([1, 512], F32, tag="smf")
                    nc.scalar.activation(
                        smf[:], p[HD : HD + 1, :],
                        mybir.ActivationFunctionType.Copy,
                    )
                    rcf = op_.tile([1, 512], F32, tag="rcf")
                    nc.vector.reciprocal_approx_fast(rcf[:], smf[:])
                    rb = op_.tile([HD, 512], F32, tag="rb")
                    nc.gpsimd.partition_broadcast(rb[:], rcf[:])
                    ot = op_.tile([HD, 512], F32, tag="ot")
                    nc.vector.tensor_tensor(
                        ot[:], p[0:HD, :], rb[:], op=mybir.AluOpType.mult
                    )
                    nc.sync.dma_start(
                        out=oT[
                            h * HD : (h + 1) * HD, half * 512 : (half + 1) * 512
                        ],
                        in_=ot[:],
                    )

            for i in range(len(items)):
                emit_front(i)
                if i >= DEPTH:
                    emit_back(i - DEPTH)
            for i in range(len(items) - DEPTH, len(items)):
                emit_back(i)

    # Bacc defers register allocation to its compile() pass, which only runs
    # in finalize(); run_bass_via_pjrt ships the BIR as-is, so finalize here.
    nc.finalize()
    return nc


def shard_inputs(hidden_states, bias, Wqkv_w, Wqkv_b):
    """Slice + lay out the full inputs into 8 per-core input maps."""
    import ml_dtypes

    bf16 = ml_dtypes.bfloat16
    hidden_states = np.asarray(hidden_states, dtype=np.float32)
    bias = np.asarray(bias, dtype=np.float32)
    Wqkv_w = np.asarray(Wqkv_w, dtype=np.float32)
    Wqkv_b = np.asarray(Wqkv_b, dtype=np.float32)

    in_maps = []
    eye = np.eye(128, dtype=bf16)
    for c in range(N_CORES):
        b, hs = c // 2, (c % 2) * HPC
        rows = np.concatenate(
            [np.arange(g * D + hs * HD, g * D + (hs + HPC) * HD) for g in range(3)]
        )
        wb2 = Wqkv_b[rows][None, :].astype(bf16)
        wbp2 = np.ascontiguousarray(
            Wqkv_b[rows].reshape(12, 128).T
        ).astype(np.float32)
        in_maps.append(
            {
                "hw": np.concatenate(
                    [hidden_states[b].T, Wqkv_w[rows].T], axis=1
                ).astype(bf16),
                "wb": wb2,
                "wbp": wbp2,
                "bT": np.ascontiguousarray(
                    bias[b, hs : hs + HPC].transpose(0, 2, 1)
                ).astype(bf16),
                "idm": eye,
            }
        )
    return in_maps


_CACHED_NC = None


def kernel(hidden_states, bias, Wqkv_w, Wqkv_b):
    from concourse.bass_utils import run_bass_kernel_spmd

    global _CACHED_NC
    if _CACHED_NC is None:
        _CACHED_NC = build_bass()
    in_maps = shard_inputs(hidden_states, bias, Wqkv_w, Wqkv_b)
    res = run_bass_kernel_spmd(_CACHED_NC, in_maps, core_ids=list(range(N_CORES)))
    out = np.empty((B, S, D), dtype=np.float32)
    for c in range(N_CORES):
        b, hs = c // 2, (c % 2) * HPC
        out[b, :, hs * HD : (hs + HPC) * HD] = res.results[c]["oT"].T
    return out
